# revision 72
# baseline (speedup 1.0000x reference)
"""GAT attention layer (nn_AttentionLayer) on 8 Trainium2 NeuronCores.

Row-sharded outputs: core c owns output rows I_c = [c*N/8, (c+1)*N/8).
Host-side staging (same values, layout/precision choices only): the j axis
is rolled per core so I_c's columns come first, then
    adjP[q] = rolled adj-block.T * 40                 fp16 {0,40}
    featP   = rolled features.T [NXC, 128, nk, jxc]   fp16
(no separate local-features input: it's featP chunk 0's first rl columns).

Device pipeline per 512-row j-quad (j on partitions, local i free),
software-pipelined so DVE/ACT/PE overlap across quads — per iteration q the
DVE stream is [TT-add(q+1), dve-tile(q), bits(q), s2(q+1,q+2)], the ACT
stream [prelus(q), hs-copy(q+2|3)], the PE stream [hs-mm(q+3), pso-mm(q)]:
    DVE:  w(q+1) = DMA(adjP[q+1]) + s1_bcast          (TT 2x, issued early
          so DVE runs while ACT does quad q's prelus)
    DVE:  dve-tile: exp(leaky(x)) = max(e^x, e^.2x) as two affine
          Schraudolph bit patterns (2 TS + TT-max), on even quads to
          balance the ACT engine
    ACT:  y = prelu(w + (s2_j-40), 0.2) per tile      (bias per tile)
    DVE:  bits = int16(y*A + B) one TS over ACT tiles (Schraudolph exp:
                                                       bitcast fp16 ~= exp(y))
    PE:   psum[c,i] += hs[j,c].T @ bits.as_fp16[j,i]  (hs as PE weights)
    out = elu(num/den) after a PE transpose of the [67, rl] accumulator;
    the tail quad runs per-tile (split TT, all-ACT, per-tile bits) so the
    final serial chain stays short.
s1_bcast comes from one matmul with wa1-broadcast weights (wa1b.T @ X^T
replicates s1 across partitions directly).  hs lookahead is 3 so the s2
scalars (read from the ACT hs-copy) never block the next TT-add.
Masked entries: x ~= s1+s2-40 -> y ~= 0.2x -> exp(y) ~ e^-8, negligible
vs row mass; no explicit mask multiply or -inf needed.
"""

import os
import sys

for _p in ("/opt/trn_rl_repo",):
    if os.path.isdir(_p) and _p not in sys.path:
        sys.path.append(_p)

import numpy as np

import concourse.bass as bass
import concourse.bacc as bacc
import concourse.mybir as mybir
import concourse.tile as tile
from concourse import bass_utils

N, D, F = 8192, 256, 64
NCORES = 8
RL = N // NCORES
BIG = 40.0
ALPHA = 0.2
# Schraudolph fp16 exp: bitcast_f16(int16(A*y + B)) ~= e^y, A = 2^10/ln2,
# B = 15360 - 61 (61 centers the mantissa-linearization error).
SCH_A = 1477.3196
SCH_B = 15299.0
RING = 8                  # adj slot ring / DMA prefetch distance (quads)
DVE_LEAKY = 8             # max j-tiles whose leaky+sch runs on DVE
BITS_ON_GPSIMD = False    # Pool TS is ~3x slower + blocks the DMA FIFO

f32 = mybir.dt.float32
fp16 = mybir.dt.float16
i16 = mybir.dt.int16
Alu = mybir.AluOpType
Act = mybir.ActivationFunctionType

LAST_RESULTS = None
_CACHE = {}


def _kernel_body(tc, out_d, featP_d, adjP_d, W_d, a_d, idn_d, n=N, rl=RL):
    nc = tc.nc
    nit = rl // 128           # local i-tiles
    njt = n // 128            # global j-tiles
    nk = D // 128             # d contraction tiles
    QT = 4                    # j-tiles per chain quad
    NQ = njt // QT
    HC = F + 3                # hs cols: h(64) | s1 | s2 | ones
    NXC = 4                   # X^T streamed in chunks along j
    jxc = n // NXC

    with (
        tc.tile_pool(name="sbP", bufs=1) as sbP,
        tc.tile_pool(name="sbS", bufs=4) as sbS,
        tc.tile_pool(name="sbA", bufs=RING) as sbA,
        tc.tile_pool(name="sbL", bufs=3) as sbL,
        tc.tile_pool(name="sbE", bufs=4) as sbE,
        tc.tile_pool(name="pp", bufs=6, space="PSUM") as pp,
        tc.tile_pool(name="pacc", bufs=1, space="PSUM") as pacc,
    ):
        aq = [
            sbA.tile([128, QT, rl], fp16, tag="aq", name=f"aq{q}")
            for q in range(min(RING, NQ))
        ]

        # ---- prologue feeds: one SWDGE FIFO in dependency order -----------
        # Host rolls the j axis per core so the local i-block is the FIRST
        # rl columns of chunk 0: s1row and the first psh blocks need only a
        # small early slice, and featTl/xTl disappears entirely.
        xTf = [
            sbS.tile([128, nk, jxc], fp16, tag="xTf", name=f"xTf{i}")
            for i in range(NXC)
        ]
        # the local-block X^T slice is split across the sync HWDGE queue and
        # the SWDGE queue so both transfer in parallel with the small loads
        hr = rl // 2
        nc.sync.dma_start(xTf[0][:, :, :hr], featP_d[0][:, :, :hr])
        nc.gpsimd.dma_start(xTf[0][:, :, hr:rl], featP_d[0][:, :, hr:rl])
        arow = sbP.tile([1, 2 * F], f32)
        nc.gpsimd.dma_start(arow[:], a_d.rearrange("f o -> o f"))
        wsb = sbP.tile([128, nk, F], f32)
        nc.gpsimd.dma_start(wsb[:], W_d.rearrange("(k p) f -> p k f", p=128))
        idn = sbP.tile([128, 128], f32)
        nc.sync.dma_start(idn[:], idn_d)

        nc.gpsimd.dma_start(aq[0][:], adjP_d[0])
        if rl < jxc:
            nc.gpsimd.dma_start(xTf[0][:, :, rl:], featP_d[0][:, :, rl:])
        if NQ > 1:
            nc.gpsimd.dma_start(aq[1][:], adjP_d[1])
        xtf_next = 1
        for q0 in range(2, min(RING, NQ)):
            nc.gpsimd.dma_start(aq[q0][:], adjP_d[q0])
            if q0 in (2, 5) and xtf_next < NXC:
                nc.gpsimd.dma_start(xTf[xtf_next][:], featP_d[xtf_next])
                xtf_next += 1
        while xtf_next < min(NXC, 3 if NQ > RING else NXC):
            nc.gpsimd.dma_start(xTf[xtf_next][:], featP_d[xtf_next])
            xtf_next += 1

        # ---- constants / rhs16 --------------------------------------------
        onesf = sbP.tile([1, 128], f32)
        nc.vector.memset(onesf[:], 1.0)
        ab = sbP.tile([128, 2 * F], f32)
        psab = pp.tile([128, 2 * F], f32, tag="big", name="psab")
        nc.tensor.matmul(psab[:], onesf[:], arow[:])
        nc.vector.tensor_copy(ab[:], psab[:])
        wa = sbP.tile([128, nk, 2], f32)
        scr = sbP.tile([128, F], f32)
        for k in range(nk):
            nc.vector.scalar_tensor_tensor(
                scr[:], wsb[:, k, :], 1.0, ab[:, :F], Alu.mult, Alu.mult,
                accum_out=wa[:, k, 0:1],
            )
            nc.vector.scalar_tensor_tensor(
                scr[:], wsb[:, k, :], 1.0, ab[:, F:], Alu.mult, Alu.mult,
                accum_out=wa[:, k, 1:2],
            )
        rhs16 = sbP.tile([128, nk, F + 2], fp16)
        for k in range(nk):
            nc.vector.tensor_copy(rhs16[:, k, :F], wsb[:, k, :])
            nc.vector.tensor_copy(rhs16[:, k, F : F + 2], wa[:, k, :])

        # ---- fast s1: s1 broadcast over partitions in one matmul step -----
        # wa1b[d, p] = wa1[d] for all p, so wa1b.T @ X^T = s1 row replicated
        # across all 128 partitions directly (no [1,rl] intermediate).
        wa1b = sbP.tile([128, nk, 128], fp16)
        for k in range(nk):
            nc.vector.tensor_scalar(
                wa1b[:, k, :],
                wa[:, k, 0:1].broadcast_to([128, 128]),
                1.0, None, Alu.mult,
            )
        s1b1 = sbP.tile([128, rl], fp16)
        for h in range(0, rl, 512):
            psb = pp.tile([128, 512], f32, tag="big", name=f"psb{h}")
            for k in range(nk):
                nc.tensor.matmul(
                    psb[:], wa1b[:, k, :], xTf[0][:, k, h : h + 512],
                    start=(k == 0), stop=(k == nk - 1),
                )
            nc.vector.tensor_copy(s1b1[:, h : h + 512], psb[:])
        s1b4 = (
            s1b1[:].rearrange("p (o i) -> p o i", o=1).broadcast_to([128, QT, rl])
        )

        # ---- hs blocks + s2 scalars ---------------------------------------
        hs_all = sbP.tile([128, njt, HC], fp16)
        nc.vector.memset(hs_all[:, :, F + 2 : F + 3], 1.0)
        # s2 - 40, per-partition scalars per j-tile (fp32), and the same
        # pre-scaled for the DVE bits path: sb1 = A*(s2-40)+B, sb2 = .2A*(..)
        s2mB = sbP.tile([128, njt], f32)
        s2b1 = sbP.tile([128, njt], f32)
        s2b2 = sbP.tile([128, njt], f32)

        psh_of = {}

        def hs_mm(g):
            # NOTE: one quad per PSUM tile — a [2,QT,66] pair tile puts the
            # last matmul output across a 2KB bank boundary (corrupts)
            psh = pp.tile([128, QT, F + 2], f32, tag="big", name=f"psh{g}")
            psh_of[g] = psh
            for tt in range(QT):
                t = g * QT + tt
                cx, ct = divmod(t, jxc // 128)
                for k in range(nk):
                    nc.tensor.matmul(
                        psh[:, tt, :], xTf[cx][:, k, ct * 128 : (ct + 1) * 128],
                        rhs16[:, k, :],
                        start=(k == 0), stop=(k == nk - 1),
                    )

        def hs_copy(g):
            psh = psh_of.pop(g)
            nc.scalar.copy(hs_all[:, g * QT : (g + 1) * QT, : F + 2], psh[:])

        def s2_block2(g, span=2):
            # s2 scalars for quads [g, g+span) in one batch (fewer DVE ops)
            ghi = min(g + span, NQ)
            sl = slice(g * QT, ghi * QT)
            s2c = hs_all[:, sl, F + 1]
            nc.vector.tensor_scalar(s2mB[:, sl], s2c, -BIG, None, Alu.add)
            nc.vector.tensor_scalar(
                s2b1[:, sl], s2c, SCH_A, SCH_B - BIG * SCH_A, Alu.mult, Alu.add
            )
            nc.vector.tensor_scalar(
                s2b2[:, sl], s2c, ALPHA * SCH_A,
                SCH_B - BIG * ALPHA * SCH_A, Alu.mult, Alu.add,
            )

        lt = [
            sbL.tile([128, QT, rl], i16, tag="lt", name=f"lt{q}") for q in range(NQ)
        ]
        labs = sbP.tile([128, rl], fp16)
        labs2 = sbP.tile([128, rl], fp16)
        # hs-as-weights accumulator: [HC, rl] += hs[:,t,:].T @ P[:, i]
        pso = pacc.tile([HC, rl], f32, name="pso")

        def tt_add(q):
            w = aq[q % RING]
            if q == NQ - 1:
                # tail quad: per-tile adds so its first prelu starts ~3 tile
                # times earlier
                for tt in range(QT):
                    nc.vector.tensor_tensor(
                        w[:, tt, :], w[:, tt, :], s1b1[:], Alu.add
                    )
            else:
                nc.vector.tensor_tensor(w[:], w[:], s1b4, Alu.add)

        HS_AHEAD = 3
        for g in range(min(2, NQ)):
            hs_mm(g)
            hs_copy(g)
        tt_add(0)
        s2_block2(0)

        ndve = 0
        for q in range(NQ):
            w = aq[q % RING]
            # PE: upcoming hs blocks' matmuls first (data is ready early);
            # lookahead 3 so the s2 scalars never wait on this iteration's
            # ACT copy (which sits behind the prelus)
            hs_gs = (2, 3) if q == 0 else (q + HS_AHEAD,)
            for g in hs_gs:
                if 1 < g < NQ:
                    hs_mm(g)
            # DVE: next quad's mask+s1 add before this quad's bits work, so
            # DVE runs while ACT does this quad's prelus (cross-quad overlap)
            if q + 1 < NQ:
                tt_add(q + 1)
            if q == NQ - 1:
                # tail quad: all tiles on ACT with per-tile bits, so PE gets
                # a steady MM stream and the final serial chain stays short
                dve_set = ()
            elif q % 2 == 0 and ndve < DVE_LEAKY:
                # one DVE-path tile per even quad; both shifting these to
                # late-quad prelus (extends ACT's critical drain) and
                # shedding early ones (early region is DMA-paced, DVE just
                # idles) measured worse — this mix is the empirical optimum
                dve_set = (QT - 1,)
            else:
                dve_set = ()
            ndve += len(dve_set)

            def dve_tile(tt):
                # leaky+exp on DVE via bits-space identity:
                # exp(leaky(x)) = max(e^x, e^.2x) -> max of two affine
                # Schraudolph bit patterns; two TS + one TT-max.
                t = q * QT + tt
                xx = w[:, tt, :]
                nc.vector.tensor_scalar(
                    labs[:], xx, SCH_A, s2b1[:, t : t + 1], Alu.mult, Alu.add
                )
                nc.vector.tensor_scalar(
                    labs2[:], xx, ALPHA * SCH_A, s2b2[:, t : t + 1],
                    Alu.mult, Alu.add,
                )
                nc.vector.tensor_tensor(
                    lt[q][:, tt, :], labs[:], labs2[:], Alu.max
                )

            def bits_tile(tt):
                nc.vector.tensor_scalar(
                    lt[q][:, tt, :], w[:, tt, :], SCH_A, SCH_B,
                    Alu.mult, Alu.add,
                )

            if q != NQ - 1:
                for tt in dve_set:
                    dve_tile(tt)
            # ACT: prelu tiles for this quad (in place; bias = s2_j - 40)
            for tt in range(QT):
                if tt in dve_set:
                    continue
                t = q * QT + tt
                nc.scalar.activation(
                    w[:, tt, :], w[:, tt, :], Act.Prelu,
                    bias=s2mB[:, t : t + 1], alpha=ALPHA,
                )
            # DVE: Schraudolph bits for the ACT tiles
            if q == NQ - 1:
                for tt in range(QT):
                    bits_tile(tt)
            else:
                nb = QT - len(dve_set)
                ltf = (lt[q][:] if nb == QT else lt[q][:, :nb, :]).rearrange(
                    "p t i -> p (t i)"
                )
                yf = (w[:] if nb == QT else w[:, :nb, :]).rearrange(
                    "p t i -> p (t i)"
                )
                eng = nc.gpsimd if BITS_ON_GPSIMD else nc.vector
                eng.tensor_scalar(ltf, yf, SCH_A, SCH_B, Alu.mult, Alu.add)
            # ACT: hs copies after this quad's prelus
            for g in hs_gs:
                if 1 < g < NQ:
                    hs_copy(g)
            # DVE: s2 scalars for the even-aligned pair (q+1, q+2); both hs
            # copies exist by now (q+1 copied at iter q-1, q+2 just above)
            if q % 2 == 1 and q + 1 < NQ:
                s2_block2(q + 1)
            w16 = lt[q][:].bitcast(fp16)
            for tt in range(QT):
                t = q * QT + tt
                for hh in range(0, rl, 512):
                    nc.tensor.matmul(
                        pso[:, hh : hh + 512], hs_all[:, t, :],
                        w16[:, tt, hh : hh + 512],
                        start=(t == 0), stop=(t == njt - 1),
                    )
            # staggered adj prefetch (ring slot reuse bounds the distance)
            if q + RING < NQ:
                nc.gpsimd.dma_start(aq[(q + RING) % RING][:], adjP_d[q + RING])
            if q == 0 and NQ > RING:
                nc.gpsimd.dma_start(xTf[3][:], featP_d[3])

        # ---- epilogue: PE-transpose pso chunks, batched divide + elu ------
        psof = sbE.tile([HC, rl], f32, tag="psof", bufs=1)
        pall = sbE.tile([128, nit, HC], f32, tag="pall", bufs=1)
        for g4 in range(0, nit, 4):
            gw = min(4, nit - g4)
            # per-group PSUM->SBUF copy overlaps the previous group's
            # transposes
            nc.vector.tensor_copy(
                psof[:, g4 * 128 : (g4 + gw) * 128],
                pso[:, g4 * 128 : (g4 + gw) * 128],
            )
            pst = pp.tile([128, 4, HC], f32, tag="big", name=f"pst{g4}")
            for i4 in range(gw):
                it = g4 + i4
                nc.tensor.transpose(
                    pst[:, i4, :], psof[:, it * 128 : (it + 1) * 128],
                    idn[:HC, :HC],
                )
            nc.vector.tensor_copy(pall[:, g4 : g4 + gw, :], pst[:, :gw, :])
        rcpa = sbE.tile([128, nit], f32, tag="rcpa", bufs=1)
        nc.vector.reciprocal(rcpa[:], pall[:, :, F + 2])
        # one broadcast multiply instead of nit per-tile tensor_scalars
        rb = (
            rcpa[:].rearrange("p (t o) -> p t o", o=1).broadcast_to([128, nit, F])
        )
        o = sbE.tile([128, nit, F], f32, tag="o", bufs=1)
        nc.vector.tensor_tensor(o[:], pall[:, :, :F], rb, Alu.mult)
        of = o[:].rearrange("p t f -> p (t f)")
        q2 = sbE.tile([128, nit * F], f32, tag="q2", bufs=1)
        e = sbE.tile([128, nit * F], f32, tag="e", bufs=1)
        r = sbE.tile([128, nit * F], f32, tag="r", bufs=1)
        # quarters: the final drain waits on the LAST chunk's DMA, so a
        # smaller last chunk issues it earlier (prior transfers overlap)
        half = max((nit * F) // 4, F)
        for hh in range(0, nit * F, half):
            sl = slice(hh, hh + half)
            nc.vector.tensor_scalar_min(q2[:, sl], of[:, sl], 0.0)
            nc.scalar.activation(e[:, sl], q2[:, sl], Act.Exp)
            nc.vector.tensor_scalar_max(r[:, sl], of[:, sl], 0.0)
            nc.vector.scalar_tensor_tensor(
                e[:, sl], e[:, sl], -1.0, r[:, sl], Alu.add, Alu.add
            )
            # contiguous per-partition store; host untiles [128, nit, F]
            nc.sync.dma_start(out_d[:, sl], e[:, sl])


def _build(n=N, rl=RL, ncores=NCORES):
    key = (n, rl, ncores)
    if key in _CACHE:
        return _CACHE[key]
    nc = bacc.Bacc(
        "TRN2", target_bir_lowering=False, debug=False, num_devices=ncores
    )
    njt = n // 128
    NQ = njt // 4
    jxc = n // 4
    nk = D // 128
    nit = rl // 128
    featP = nc.dram_tensor("featP", [4, 128, nk, jxc], fp16, kind="ExternalInput").ap()
    adjP = nc.dram_tensor("adjP", [NQ, 128, 4, rl], fp16, kind="ExternalInput").ap()
    W = nc.dram_tensor("W", [D, F], f32, kind="ExternalInput").ap()
    a = nc.dram_tensor("a", [2 * F, 1], f32, kind="ExternalInput").ap()
    idn = nc.dram_tensor("idn", [128, 128], f32, kind="ExternalInput").ap()
    out = nc.dram_tensor("out", [128, nit * F], f32, kind="ExternalOutput").ap()
    with tile.TileContext(nc) as tc:
        _kernel_body(tc, out, featP, adjP, W, a, idn, n=n, rl=rl)
    nc.compile()
    _CACHE[key] = nc
    return nc


def kernel(features, adj, W, a):
    global LAST_RESULTS
    features = np.ascontiguousarray(features, dtype=np.float32)
    adj = np.ascontiguousarray(adj, dtype=np.int32)
    W = np.ascontiguousarray(W, dtype=np.float32)
    a = np.ascontiguousarray(a, dtype=np.float32)

    n = adj.shape[0]
    rl = n // NCORES
    nit = rl // 128
    njt = n // 128
    NQ = njt // 4
    nk = D // 128
    jxc = n // 4
    nc = _build(n=n, rl=rl, ncores=NCORES)
    # Per core, roll the j axis so the core's own i-block comes first:
    # j_new -> j_old = (j_new + c*rl) mod n.  Softmax sums are j-order
    # invariant, so only featP and adjP need the consistent roll.
    fT16 = features.T.astype(np.float16)                    # [D, n]
    idn = np.eye(128, dtype=np.float32)
    in_maps = []
    for c in range(NCORES):
        fT16c = np.roll(fT16, -c * rl, axis=1)
        # featP[cx, p, k, j] = fT16c[(k*128+p), cx*jxc + j]
        featP = np.ascontiguousarray(
            fT16c.reshape(nk, 128, 4, jxc).transpose(2, 1, 0, 3)
        )
        adjT40 = adj[c * rl : (c + 1) * rl].T.astype(np.float16) * np.float16(BIG)
        adjT40 = np.roll(adjT40, -c * rl, axis=0)
        # adjP[Q, p, t, i] = adjT40[Q*512 + t*128 + p, i]
        adjP = adjT40.reshape(NQ, 4, 128, rl).transpose(0, 2, 1, 3).copy()
        in_maps.append(
            {
                "featP": featP,
                "adjP": np.ascontiguousarray(adjP),
                "W": W,
                "a": a,
                "idn": idn,
            }
        )
    res = bass_utils.run_bass_kernel_spmd(nc, in_maps, core_ids=list(range(NCORES)))
    LAST_RESULTS = res
    return np.concatenate(
        [
            res.results[c]["out"]
            .reshape(128, nit, F)
            .transpose(1, 0, 2)
            .reshape(rl, F)
            for c in range(NCORES)
        ],
        axis=0,
    )


# revision 73
# speedup vs baseline: 1.0431x; 1.0431x over previous
"""GAT attention layer (nn_AttentionLayer) on 8 Trainium2 NeuronCores.

Row-sharded outputs: core c owns output rows I_c = [c*N/8, (c+1)*N/8).
Host-side staging (same values, layout/precision choices only): the j axis
is rolled per core so I_c's columns come first, then
    adjP[q] = rolled adj-block.T * 40                 fp16 {0,40}
    featP   = rolled features.T [NXC, 128, nk, jxc]   fp16
(no separate local-features input: it's featP chunk 0's first rl columns).

Device pipeline per 512-row j-quad (j on partitions, local i free),
software-pipelined so DVE/ACT/PE overlap across quads — per iteration q the
DVE stream is [TT-add(q+1), dve-tile(q), bits(q), s2(q+1,q+2)], the ACT
stream [prelus(q), hs-copy(q+2|3)], the PE stream [hs-mm(q+3), pso-mm(q)]:
    DVE:  w(q+1) = DMA(adjP[q+1]) + s1_bcast          (TT 2x, issued early
          so DVE runs while ACT does quad q's prelus)
    DVE:  dve-tile: exp(leaky(x)) = max(e^x, e^.2x) as two affine
          Schraudolph bit patterns (2 TS + TT-max), on even quads to
          balance the ACT engine
    ACT:  y = prelu(w + (s2_j-40), 0.2) per tile      (bias per tile)
    DVE:  bits = int16(y*A + B) one TS over ACT tiles (Schraudolph exp:
                                                       bitcast fp16 ~= exp(y))
    PE:   psum[c,i] += hs[j,c].T @ bits.as_fp16[j,i]  (hs as PE weights)
    out = elu(num/den) after a PE transpose of the [67, rl] accumulator;
    the tail quad runs per-tile (split TT, all-ACT, per-tile bits) so the
    final serial chain stays short.
s1_bcast comes from one matmul with wa1-broadcast weights (wa1b.T @ X^T
replicates s1 across partitions directly).  hs lookahead is 3 so the s2
scalars (read from the ACT hs-copy) never block the next TT-add.
Masked entries: x ~= s1+s2-40 -> y ~= 0.2x -> exp(y) ~ e^-8, negligible
vs row mass; no explicit mask multiply or -inf needed.
"""

import os
import sys

for _p in ("/opt/trn_rl_repo",):
    if os.path.isdir(_p) and _p not in sys.path:
        sys.path.append(_p)

import numpy as np

import concourse.bass as bass
import concourse.bacc as bacc
import concourse.mybir as mybir
import concourse.tile as tile
from concourse import bass_utils

N, D, F = 8192, 256, 64
NCORES = 8
RL = N // NCORES
BIG = 40.0
ALPHA = 0.2
# Schraudolph fp16 exp: bitcast_f16(int16(A*y + B)) ~= e^y, A = 2^10/ln2,
# B = 15360 - 61 (61 centers the mantissa-linearization error).
SCH_A = 1477.3196
SCH_B = 15299.0
RING = 8                  # adj slot ring / DMA prefetch distance (quads)
DVE_LEAKY = 8             # max j-tiles whose leaky+sch runs on DVE
BITS_ON_GPSIMD = False    # Pool TS is ~3x slower + blocks the DMA FIFO

f32 = mybir.dt.float32
fp16 = mybir.dt.float16
i16 = mybir.dt.int16
Alu = mybir.AluOpType
Act = mybir.ActivationFunctionType

LAST_RESULTS = None
_CACHE = {}


def _kernel_body(tc, out_d, featP_d, adjP_d, W_d, a_d, idn_d, n=N, rl=RL):
    nc = tc.nc
    nit = rl // 128           # local i-tiles
    njt = n // 128            # global j-tiles
    nk = D // 128             # d contraction tiles
    QT = 4                    # j-tiles per chain quad
    NQ = njt // QT
    HC = F + 3                # hs cols: h(64) | s1 | s2 | ones
    NXC = 4                   # X^T streamed in chunks along j
    jxc = n // NXC

    with (
        tc.tile_pool(name="sbP", bufs=1) as sbP,
        tc.tile_pool(name="sbS", bufs=4) as sbS,
        tc.tile_pool(name="sbA", bufs=RING) as sbA,
        tc.tile_pool(name="sbL", bufs=3) as sbL,
        tc.tile_pool(name="sbE", bufs=4) as sbE,
        tc.tile_pool(name="pp", bufs=6, space="PSUM") as pp,
        tc.tile_pool(name="pacc", bufs=1, space="PSUM") as pacc,
    ):
        aq = [
            sbA.tile([128, QT, rl], fp16, tag="aq", name=f"aq{q}")
            for q in range(min(RING, NQ))
        ]

        # ---- prologue feeds: one SWDGE FIFO in dependency order -----------
        # Host rolls the j axis per core so the local i-block is the FIRST
        # rl columns of chunk 0: s1row and the first psh blocks need only a
        # small early slice, and featTl/xTl disappears entirely.
        xTf = [
            sbS.tile([128, nk, jxc], fp16, tag="xTf", name=f"xTf{i}")
            for i in range(NXC)
        ]
        # the local-block X^T slice is split across the sync HWDGE queue and
        # the SWDGE queue so both transfer in parallel with the small loads
        hr = rl // 2
        nc.sync.dma_start(xTf[0][:, :, :hr], featP_d[0][:, :, :hr])
        nc.gpsimd.dma_start(xTf[0][:, :, hr:rl], featP_d[0][:, :, hr:rl])
        arow = sbP.tile([1, 2 * F], f32)
        nc.gpsimd.dma_start(arow[:], a_d.rearrange("f o -> o f"))
        wsb = sbP.tile([128, nk, F], f32)
        nc.gpsimd.dma_start(wsb[:], W_d.rearrange("(k p) f -> p k f", p=128))
        idn = sbP.tile([128, 128], f32)
        nc.sync.dma_start(idn[:], idn_d)

        nc.gpsimd.dma_start(aq[0][:], adjP_d[0])
        if rl < jxc:
            nc.gpsimd.dma_start(xTf[0][:, :, rl:], featP_d[0][:, :, rl:])
        if NQ > 1:
            nc.gpsimd.dma_start(aq[1][:], adjP_d[1])
        xtf_next = 1
        for q0 in range(2, min(RING, NQ)):
            nc.gpsimd.dma_start(aq[q0][:], adjP_d[q0])
            if q0 in (2, 5) and xtf_next < NXC:
                nc.gpsimd.dma_start(xTf[xtf_next][:], featP_d[xtf_next])
                xtf_next += 1
        while xtf_next < min(NXC, 3 if NQ > RING else NXC):
            nc.gpsimd.dma_start(xTf[xtf_next][:], featP_d[xtf_next])
            xtf_next += 1

        # ---- constants / rhs16 --------------------------------------------
        onesf = sbP.tile([1, 128], f32)
        nc.vector.memset(onesf[:], 1.0)
        ab = sbP.tile([128, 2 * F], f32)
        psab = pp.tile([128, 2 * F], f32, tag="big", name="psab")
        nc.tensor.matmul(psab[:], onesf[:], arow[:])
        nc.vector.tensor_copy(ab[:], psab[:])
        wa = sbP.tile([128, nk, 2], f32)
        scr = sbP.tile([128, F], f32)
        for k in range(nk):
            nc.vector.scalar_tensor_tensor(
                scr[:], wsb[:, k, :], 1.0, ab[:, :F], Alu.mult, Alu.mult,
                accum_out=wa[:, k, 0:1],
            )
            nc.vector.scalar_tensor_tensor(
                scr[:], wsb[:, k, :], 1.0, ab[:, F:], Alu.mult, Alu.mult,
                accum_out=wa[:, k, 1:2],
            )
        rhs16 = sbP.tile([128, nk, F + 2], fp16)
        for k in range(nk):
            nc.vector.tensor_copy(rhs16[:, k, :F], wsb[:, k, :])
            nc.vector.tensor_copy(rhs16[:, k, F : F + 2], wa[:, k, :])

        # ---- fast s1: s1 broadcast over partitions in one matmul step -----
        # wa1b[d, p] = wa1[d] for all p, so wa1b.T @ X^T = s1 row replicated
        # across all 128 partitions directly (no [1,rl] intermediate).
        wa1b = sbP.tile([128, nk, 128], fp16)
        for k in range(nk):
            nc.vector.tensor_scalar(
                wa1b[:, k, :],
                wa[:, k, 0:1].broadcast_to([128, 128]),
                1.0, None, Alu.mult,
            )
        s1b1 = sbP.tile([128, rl], fp16)
        for h in range(0, rl, 512):
            psb = pp.tile([128, 512], f32, tag="big", name=f"psb{h}")
            for k in range(nk):
                nc.tensor.matmul(
                    psb[:], wa1b[:, k, :], xTf[0][:, k, h : h + 512],
                    start=(k == 0), stop=(k == nk - 1),
                )
            nc.vector.tensor_copy(s1b1[:, h : h + 512], psb[:])
        s1b4 = (
            s1b1[:].rearrange("p (o i) -> p o i", o=1).broadcast_to([128, QT, rl])
        )

        # ---- hs blocks + s2 scalars ---------------------------------------
        hs_all = sbP.tile([128, njt, HC], fp16)
        nc.vector.memset(hs_all[:, :, F + 2 : F + 3], 1.0)
        # s2 - 40, per-partition scalars per j-tile (fp32), and the same
        # pre-scaled for the DVE bits path: sb1 = A*(s2-40)+B, sb2 = .2A*(..)
        s2mB = sbP.tile([128, njt], f32)
        s2b1 = sbP.tile([128, njt], f32)
        s2b2 = sbP.tile([128, njt], f32)

        psh_of = {}

        def hs_mm(g):
            # NOTE: one quad per PSUM tile — a [2,QT,66] pair tile puts the
            # last matmul output across a 2KB bank boundary (corrupts)
            psh = pp.tile([128, QT, F + 2], f32, tag="big", name=f"psh{g}")
            psh_of[g] = psh
            for tt in range(QT):
                t = g * QT + tt
                cx, ct = divmod(t, jxc // 128)
                for k in range(nk):
                    nc.tensor.matmul(
                        psh[:, tt, :], xTf[cx][:, k, ct * 128 : (ct + 1) * 128],
                        rhs16[:, k, :],
                        start=(k == 0), stop=(k == nk - 1),
                    )

        def hs_copy(g):
            psh = psh_of.pop(g)
            nc.scalar.copy(hs_all[:, g * QT : (g + 1) * QT, : F + 2], psh[:])

        def s2_block2(g, span=2):
            # s2 scalars for quads [g, g+span) in one batch (fewer DVE ops)
            ghi = min(g + span, NQ)
            sl = slice(g * QT, ghi * QT)
            s2c = hs_all[:, sl, F + 1]
            nc.vector.tensor_scalar(s2mB[:, sl], s2c, -BIG, None, Alu.add)
            nc.vector.tensor_scalar(
                s2b1[:, sl], s2c, SCH_A, SCH_B - BIG * SCH_A, Alu.mult, Alu.add
            )
            nc.vector.tensor_scalar(
                s2b2[:, sl], s2c, ALPHA * SCH_A,
                SCH_B - BIG * ALPHA * SCH_A, Alu.mult, Alu.add,
            )

        lt = [
            sbL.tile([128, QT, rl], i16, tag="lt", name=f"lt{q}") for q in range(NQ)
        ]
        labs = sbP.tile([128, rl], fp16)
        labs2 = sbP.tile([128, rl], fp16)
        # hs-as-weights accumulator: [HC, rl] += hs[:,t,:].T @ P[:, i]
        pso = pacc.tile([HC, rl], f32, name="pso")

        def tt_add(q):
            w = aq[q % RING]
            if q == NQ - 1:
                # tail quad: per-tile adds so its first prelu starts ~3 tile
                # times earlier
                for tt in range(QT):
                    nc.vector.tensor_tensor(
                        w[:, tt, :], w[:, tt, :], s1b1[:], Alu.add
                    )
            else:
                nc.vector.tensor_tensor(w[:], w[:], s1b4, Alu.add)

        HS_AHEAD = 3
        for g in range(min(2, NQ)):
            hs_mm(g)
            hs_copy(g)
        tt_add(0)
        s2_block2(0)

        ndve = 0
        for q in range(NQ):
            w = aq[q % RING]
            # PE: upcoming hs blocks' matmuls first (data is ready early);
            # lookahead 3 so the s2 scalars never wait on this iteration's
            # ACT copy (which sits behind the prelus)
            hs_gs = (2, 3) if q == 0 else (q + HS_AHEAD,)
            for g in hs_gs:
                if 1 < g < NQ:
                    hs_mm(g)
            # DVE: next quad's mask+s1 add before this quad's bits work, so
            # DVE runs while ACT does this quad's prelus (cross-quad overlap)
            if q + 1 < NQ:
                tt_add(q + 1)
            if q == NQ - 1:
                # tail quad: all tiles on ACT with per-tile bits, so PE gets
                # a steady MM stream and the final serial chain stays short
                dve_set = ()
            elif q % 2 == 0 and ndve < DVE_LEAKY:
                # one DVE-path tile per even quad; both shifting these to
                # late-quad prelus (extends ACT's critical drain) and
                # shedding early ones (early region is DMA-paced, DVE just
                # idles) measured worse — this mix is the empirical optimum
                dve_set = (QT - 1,)
            else:
                dve_set = ()
            ndve += len(dve_set)

            def dve_tile(tt):
                # leaky+exp on DVE via bits-space identity:
                # exp(leaky(x)) = max(e^x, e^.2x) -> max of two affine
                # Schraudolph bit patterns; two TS + one TT-max.
                t = q * QT + tt
                xx = w[:, tt, :]
                nc.vector.tensor_scalar(
                    labs[:], xx, SCH_A, s2b1[:, t : t + 1], Alu.mult, Alu.add
                )
                nc.vector.tensor_scalar(
                    labs2[:], xx, ALPHA * SCH_A, s2b2[:, t : t + 1],
                    Alu.mult, Alu.add,
                )
                nc.vector.tensor_tensor(
                    lt[q][:, tt, :], labs[:], labs2[:], Alu.max
                )

            def bits_tile(tt):
                nc.vector.tensor_scalar(
                    lt[q][:, tt, :], w[:, tt, :], SCH_A, SCH_B,
                    Alu.mult, Alu.add,
                )

            if q != NQ - 1:
                for tt in dve_set:
                    dve_tile(tt)
            # ACT: prelu tiles for this quad (in place; bias = s2_j - 40)
            for tt in range(QT):
                if tt in dve_set:
                    continue
                t = q * QT + tt
                nc.scalar.activation(
                    w[:, tt, :], w[:, tt, :], Act.Prelu,
                    bias=s2mB[:, t : t + 1], alpha=ALPHA,
                )
            # DVE: Schraudolph bits for the ACT tiles
            if q == NQ - 1:
                for tt in range(QT):
                    bits_tile(tt)
            else:
                nb = QT - len(dve_set)
                ltf = (lt[q][:] if nb == QT else lt[q][:, :nb, :]).rearrange(
                    "p t i -> p (t i)"
                )
                yf = (w[:] if nb == QT else w[:, :nb, :]).rearrange(
                    "p t i -> p (t i)"
                )
                eng = nc.gpsimd if BITS_ON_GPSIMD else nc.vector
                eng.tensor_scalar(ltf, yf, SCH_A, SCH_B, Alu.mult, Alu.add)
            # ACT: hs copies after this quad's prelus
            for g in hs_gs:
                if 1 < g < NQ:
                    hs_copy(g)
            # DVE: s2 scalars for the even-aligned pair (q+1, q+2); both hs
            # copies exist by now (q+1 copied at iter q-1, q+2 just above)
            if q % 2 == 1 and q + 1 < NQ:
                s2_block2(q + 1)
            w16 = lt[q][:].bitcast(fp16)
            for tt in range(QT):
                t = q * QT + tt
                for hh in range(0, rl, 512):
                    nc.tensor.matmul(
                        pso[:, hh : hh + 512], hs_all[:, t, :],
                        w16[:, tt, hh : hh + 512],
                        start=(t == 0), stop=(t == njt - 1),
                    )
            # staggered adj prefetch (ring slot reuse bounds the distance)
            if q + RING < NQ:
                nc.gpsimd.dma_start(aq[(q + RING) % RING][:], adjP_d[q + RING])
            if q == 0 and NQ > RING:
                nc.gpsimd.dma_start(xTf[3][:], featP_d[3])

        # ---- epilogue: PE-transpose pso chunks, batched divide + elu ------
        psof = sbE.tile([HC, rl], f32, tag="psof", bufs=1)
        pall = sbE.tile([128, nit, HC], f32, tag="pall", bufs=1)
        for g4 in range(0, nit, 4):
            gw = min(4, nit - g4)
            # per-group PSUM->SBUF copy overlaps the previous group's
            # transposes
            nc.vector.tensor_copy(
                psof[:, g4 * 128 : (g4 + gw) * 128],
                pso[:, g4 * 128 : (g4 + gw) * 128],
            )
            pst = pp.tile([128, 4, HC], f32, tag="big", name=f"pst{g4}")
            for i4 in range(gw):
                it = g4 + i4
                nc.tensor.transpose(
                    pst[:, i4, :], psof[:, it * 128 : (it + 1) * 128],
                    idn[:HC, :HC],
                )
            nc.vector.tensor_copy(pall[:, g4 : g4 + gw, :], pst[:, :gw, :])
        rcpa = sbE.tile([128, nit], f32, tag="rcpa", bufs=1)
        nc.vector.reciprocal(rcpa[:], pall[:, :, F + 2])
        # one broadcast multiply instead of nit per-tile tensor_scalars
        rb = (
            rcpa[:].rearrange("p (t o) -> p t o", o=1).broadcast_to([128, nit, F])
        )
        o = sbE.tile([128, nit, F], f32, tag="o", bufs=1)
        nc.vector.tensor_tensor(o[:], pall[:, :, :F], rb, Alu.mult)
        of = o[:].rearrange("p t f -> p (t f)")
        q2 = sbE.tile([128, nit * F], f32, tag="q2", bufs=1)
        e = sbE.tile([128, nit * F], f32, tag="e", bufs=1)
        r = sbE.tile([128, nit * F], f32, tag="r", bufs=1)
        half = (nit * F) // 2
        for hh in (0, half):
            sl = slice(hh, hh + half)
            nc.vector.tensor_scalar_min(q2[:, sl], of[:, sl], 0.0)
            nc.scalar.activation(e[:, sl], q2[:, sl], Act.Exp)
            nc.vector.tensor_scalar_max(r[:, sl], of[:, sl], 0.0)
            nc.vector.scalar_tensor_tensor(
                e[:, sl], e[:, sl], -1.0, r[:, sl], Alu.add, Alu.add
            )
            # contiguous per-partition store; host untiles [128, nit, F]
            nc.sync.dma_start(out_d[:, sl], e[:, sl])


def _build(n=N, rl=RL, ncores=NCORES):
    key = (n, rl, ncores)
    if key in _CACHE:
        return _CACHE[key]
    nc = bacc.Bacc(
        "TRN2", target_bir_lowering=False, debug=False, num_devices=ncores
    )
    njt = n // 128
    NQ = njt // 4
    jxc = n // 4
    nk = D // 128
    nit = rl // 128
    featP = nc.dram_tensor("featP", [4, 128, nk, jxc], fp16, kind="ExternalInput").ap()
    adjP = nc.dram_tensor("adjP", [NQ, 128, 4, rl], fp16, kind="ExternalInput").ap()
    W = nc.dram_tensor("W", [D, F], f32, kind="ExternalInput").ap()
    a = nc.dram_tensor("a", [2 * F, 1], f32, kind="ExternalInput").ap()
    idn = nc.dram_tensor("idn", [128, 128], f32, kind="ExternalInput").ap()
    out = nc.dram_tensor("out", [128, nit * F], f32, kind="ExternalOutput").ap()
    with tile.TileContext(nc) as tc:
        _kernel_body(tc, out, featP, adjP, W, a, idn, n=n, rl=rl)
    nc.compile()
    _CACHE[key] = nc
    return nc


def kernel(features, adj, W, a):
    global LAST_RESULTS
    features = np.ascontiguousarray(features, dtype=np.float32)
    adj = np.ascontiguousarray(adj, dtype=np.int32)
    W = np.ascontiguousarray(W, dtype=np.float32)
    a = np.ascontiguousarray(a, dtype=np.float32)

    n = adj.shape[0]
    rl = n // NCORES
    nit = rl // 128
    njt = n // 128
    NQ = njt // 4
    nk = D // 128
    jxc = n // 4
    nc = _build(n=n, rl=rl, ncores=NCORES)
    # Per core, roll the j axis so the core's own i-block comes first:
    # j_new -> j_old = (j_new + c*rl) mod n.  Softmax sums are j-order
    # invariant, so only featP and adjP need the consistent roll.
    fT16 = features.T.astype(np.float16)                    # [D, n]
    idn = np.eye(128, dtype=np.float32)
    in_maps = []
    for c in range(NCORES):
        fT16c = np.roll(fT16, -c * rl, axis=1)
        # featP[cx, p, k, j] = fT16c[(k*128+p), cx*jxc + j]
        featP = np.ascontiguousarray(
            fT16c.reshape(nk, 128, 4, jxc).transpose(2, 1, 0, 3)
        )
        adjT40 = adj[c * rl : (c + 1) * rl].T.astype(np.float16) * np.float16(BIG)
        adjT40 = np.roll(adjT40, -c * rl, axis=0)
        # adjP[Q, p, t, i] = adjT40[Q*512 + t*128 + p, i]
        adjP = adjT40.reshape(NQ, 4, 128, rl).transpose(0, 2, 1, 3).copy()
        in_maps.append(
            {
                "featP": featP,
                "adjP": np.ascontiguousarray(adjP),
                "W": W,
                "a": a,
                "idn": idn,
            }
        )
    res = bass_utils.run_bass_kernel_spmd(nc, in_maps, core_ids=list(range(NCORES)))
    LAST_RESULTS = res
    return np.concatenate(
        [
            res.results[c]["out"]
            .reshape(128, nit, F)
            .transpose(1, 0, 2)
            .reshape(rl, F)
            for c in range(NCORES)
        ],
        axis=0,
    )
